# revision 4
# baseline (speedup 1.0000x reference)
"""Trainium2 Bass kernel: GQA attention block (nn_Attention_66142496358763).

Full module: x -> (wq,wk,wv) projections -> RoPE(q,k) -> softmax(q k^T/sqrt(d)) v
(GQA: 32 q heads, 8 kv heads) -> wo projection.

Sharding (tensor-parallel over heads, 8 cores):
  core c: q heads [4c, 4c+4), kv head c, wq/wk/wv column shards, wo row shard
  -> each core emits a partial [S, DIM] output; host sums the 8 partials.

All TensorE math in fp16 (full-rate on trn2), fp32 PSUM accumulation,
softmax exp in fp32 on ScalarE. Softmax is computed without the max
subtraction (scores are O(10) here; a -4 bias inside exp keeps the fp16
P-matrix in range) and the denominator comes for free from a ones-column
appended to V inside the PV matmul. A/V transposes ride the DMA xbar.

Schedule: the ScalarE exp stream (~1.4us per key-tile pair) outpaces the
attention-only PE work (~0.9us per pair), so the wo-projection matmuls and
the deferred quarter-3 Q projections are interleaved into the attention
inner loop as filler, paced by a credit model so the PE never waits on the
exp->PV dependency. PV accumulators are packed two-per-PSUM-bank and
double-buffered across heads so the divide of head h never blocks head h+1.
"""

import numpy as np
from collections import deque

S = 2048
DIM = 4096
HD = 128
NCORES = 8
HPC = 4          # q heads per core
QB = 512         # q block (seq block) size
NQB = S // QB    # 4
DKT = DIM // 128  # 32 contraction tiles for projections
KT = S // 128    # 16 key tiles for attention
NPAIR = KT // 2  # 8 score/exp pairs per (head, qblock)
SCALE = float(HD) ** -0.5
EXP_BIAS = -4.0
N_WARM = 120

# credit-model constants (ns of engine time per emitted op)
ACT_PAIR_NS = 1640.0   # two 512-wide exps + semaphore handling per kp pair
PE_S_NS = 432.0        # two 512-col score matmuls
PE_PV_NS = 456.0       # eight 129-col PV matmuls
PE_MARGIN = 500.0

_CACHE = {}


def _build_nc():
    import concourse.bass as bass
    import concourse.tile as tile
    from concourse import bacc, mybir

    fp16 = mybir.dt.float16
    f32 = mybir.dt.float32
    AF = mybir.ActivationFunctionType

    nc = bacc.Bacc("TRN2", target_bir_lowering=False, debug=False)

    xt_d = nc.dram_tensor("xt", [DIM, S], fp16, kind="ExternalInput").ap()
    wq_d = nc.dram_tensor("wq", [DIM, HPC * HD], fp16, kind="ExternalInput").ap()
    wkv_d = nc.dram_tensor("wkv", [DIM, 2 * HD], fp16, kind="ExternalInput").ap()
    wo_d = nc.dram_tensor("wo", [HPC * HD, DIM], fp16, kind="ExternalInput").ap()
    rc_d = nc.dram_tensor("ropec", [HD, S], fp16, kind="ExternalInput").ap()
    rs_d = nc.dram_tensor("ropes", [HD, S], fp16, kind="ExternalInput").ap()
    sw_d = nc.dram_tensor("pswap", [HD, HD], fp16, kind="ExternalInput").ap()
    out_d = nc.dram_tensor("out", [S, DIM], fp16, kind="ExternalOutput").ap()

    xt_r = xt_d.rearrange("(kt p) s -> p kt s", p=128)
    wq_r = wq_d.rearrange("(kt p) n -> p kt n", p=128)
    wkv_r = wkv_d.rearrange("(kt p) n -> p kt n", p=128)
    wo_r = wo_d.rearrange("(h p) n -> p h n", p=128)
    out_r = out_d.rearrange("(st p) n -> st p n", p=128)

    with tile.TileContext(nc) as tc:
        with (
            tc.tile_pool(name="const", bufs=1) as const,
            tc.tile_pool(name="xtp", bufs=5) as xtp,
            tc.tile_pool(name="persist", bufs=1) as persist,
            tc.tile_pool(name="tmp", bufs=5) as tmp,
            tc.tile_pool(name="t12", bufs=4) as t12,
            tc.tile_pool(name="pt", bufs=4) as ptp,
            tc.tile_pool(name="asb", bufs=6) as asbp,
            tc.tile_pool(name="small", bufs=8) as small,
            tc.tile_pool(name="outp", bufs=2) as outp,
            tc.tile_pool(name="psum", bufs=1, space="PSUM") as psum,
        ):
            # ---- inputs split across the two HWDGE queues:
            # sync: wk + the xt chunk stream; scalar: rope consts, wv, wq, wo
            xt_tiles = {0: []}
            for j in range(4):
                t = xtp.tile([128, 8, QB], fp16, tag="xt", name=f"xt_0_{j}")
                nc.sync.dma_start(t[:], xt_r[:, 8 * j:8 * (j + 1), 0:QB])
                xt_tiles[0].append(t)
            wkv_c = []
            wq_c = []
            for j in range(4):
                w = const.tile([128, 8, 2 * HD], fp16, tag=f"c_wkv{j}", name=f"wkv_c{j}")
                nc.scalar.dma_start(w[:], wkv_r[:, 8 * j:8 * (j + 1), :])
                wkv_c.append(w)
                q = const.tile([128, 8, 512], fp16, tag=f"c_wq{j}", name=f"wq_c{j}")
                nc.scalar.dma_start(q[:], wq_r[:, 8 * j:8 * (j + 1), :])
                wq_c.append(q)
            rc_sb = const.tile([HD, S], fp16, tag="c_rc")
            nc.scalar.dma_start(rc_sb[:], rc_d[:])
            rs_sb = const.tile([HD, S], fp16, tag="c_rs")
            nc.scalar.dma_start(rs_sb[:], rs_d[:])
            sw_sb = const.tile([HD, HD], fp16, tag="c_sw")
            nc.scalar.dma_start(sw_sb[:], sw_d[:])
            ebias_sb = const.tile([128, 1], f32, tag="c_eb")
            nc.gpsimd.memset(ebias_sb[:], EXP_BIAS)

            # PE warm-up: dummy matmuls with no input deps, so the HAM
            # clock-gate opens while the first DMAs are still streaming in
            warm_sb = const.tile([128, 128], fp16, tag="c_warm")
            nc.gpsimd.memset(warm_sb[:], 0.0)
            warm_ps = psum.tile([128, 1024], f32, tag="s", name="warm_ps")
            for _ in range(N_WARM):
                nc.tensor.matmul(
                    warm_ps[:, 0:128], warm_sb[:], warm_sb[:], start=True, stop=True
                )

            # persistent activations
            qt_sb = persist.tile([128, HPC, S], fp16, tag="p_qt")   # rope'd Q^T per head
            kt_sb = persist.tile([128, S], fp16, tag="p_kt")        # rope'd K^T
            va_sb = persist.tile([128, KT, 256], fp16, tag="p_va")  # V natural + ones col (256B-aligned rows for the xbar transpose)
            at_sb = persist.tile([128, HPC, S], fp16, tag="p_at")   # A^T per head
            nc.gpsimd.memset(va_sb[:, :, 128:130], 1.0)

            wo_sb = const.tile([128, HPC, DIM], fp16, tag="c_wo")

            # ---- phase 1: projections + rope, one seq-quarter at a time ----
            # unit order per quarter: K, V, Q0..Q3; quarter 3 only projects
            # K and V here — its four Q units are deferred into the attention
            # phase as PE filler (their xt tiles are the last ones allocated
            # from the pool, so they stay resident).
            pending = []

            def finish_unit(kind, raw, q0, u):
                if kind == "v":
                    kt0 = q0 // 128
                    for j in range(4):
                        nc.scalar.dma_start_transpose(
                            va_sb[:, kt0 + j, 0:128],
                            raw[:, j * 128:(j + 1) * 128],
                        )
                else:
                    sw_ps = psum.tile([128, QB], f32, tag="s", name="sw_ps")
                    nc.tensor.matmul(sw_ps[:], sw_sb[:], raw[:], start=True, stop=True)
                    t1 = t12.tile([128, QB], fp16, tag="t12")
                    nc.vector.tensor_mul(t1[:], raw[:], rc_sb[:, q0:q0 + QB])
                    t2 = t12.tile([128, QB], fp16, tag="t12")
                    nc.vector.tensor_mul(t2[:], sw_ps[:], rs_sb[:, q0:q0 + QB])
                    if kind == "q":
                        dest = qt_sb[:, u, q0:q0 + QB]
                    else:
                        dest = kt_sb[:, q0:q0 + QB]
                    nc.vector.tensor_add(dest, t1[:], t2[:])

            def prefetch_xt(qi, j):
                if qi >= NQB:
                    return
                lst = xt_tiles.setdefault(qi, [])
                if len(lst) > j:
                    return
                nq0 = qi * QB
                t = xtp.tile([128, 8, QB], fp16, tag="xt", name=f"xt_{qi}_{j}")
                nc.sync.dma_start(t[:], xt_r[:, 8 * j:8 * (j + 1), nq0:nq0 + QB])
                lst.append(t)

            def w_slice(kind, h, kt):
                if kind == "q":
                    return wq_c[kt // 8][:, kt % 8, h * HD:(h + 1) * HD]
                if kind == "k":
                    return wkv_c[kt // 8][:, kt % 8, 0:HD]
                return wkv_c[kt // 8][:, kt % 8, HD:2 * HD]

            PS_TAGS = ["accA", "accB", "wo"]

            for qi in range(NQB):
                q0 = qi * QB
                xt_c = xt_tiles[qi]
                if qi < NQB - 1:
                    groups = [[("k", -1), ("v", -1), ("q", 0)],
                              [("q", 1), ("q", 2), ("q", 3)]]
                else:
                    groups = [[("k", -1), ("v", -1)]]
                for gi, grp in enumerate(groups):
                    if gi == 0:
                        prefetch_xt(qi + 1, 0)
                        prefetch_xt(qi + 1, 1)
                    else:
                        prefetch_xt(qi + 1, 2)
                        prefetch_xt(qi + 1, 3)
                    pss = [
                        psum.tile([128, QB], f32, tag=PS_TAGS[gu], bufs=2,
                                  name=f"pj{gi}{gu}")
                        for gu in range(len(grp))
                    ]
                    for j in range(4):
                        for gu, (kind, h) in enumerate(grp):
                            for kt in range(8 * j, 8 * j + 8):
                                nc.tensor.matmul(
                                    pss[gu][:],
                                    w_slice(kind, h, kt),
                                    xt_c[kt // 8][:, kt % 8, :],
                                    start=(kt == 0),
                                    stop=(kt == DKT - 1),
                                )
                        if pending:
                            finish_unit(*pending.pop(0))
                    for gu, (kind, h) in enumerate(grp):
                        raw = tmp.tile([128, QB], fp16, tag="tmp")
                        nc.scalar.copy(raw[:], pss[gu][:])
                        pending.append((kind, raw, q0, h))
            while pending:
                finish_unit(*pending.pop(0))

            nc.scalar.dma_start(wo_sb[:], wo_r[:])

            # ---- phase 2+3: attention with interleaved filler ----
            credits = {"pe": 0.0, "act": 0.0}

            def gen_proj():
                # deferred quarter-3 Q projections (rope'd into qt_sb)
                xt_c = xt_tiles[NQB - 1]
                q0 = (NQB - 1) * QB
                for u in range(HPC):
                    pss = psum.tile([128, QB], f32, tag="wo", bufs=2,
                                    name=f"dq{u}")
                    for j in range(4):
                        for kt in range(8 * j, 8 * j + 8):
                            nc.tensor.matmul(
                                pss[:],
                                wq_c[kt // 8][:, kt % 8, u * HD:(u + 1) * HD],
                                xt_c[kt // 8][:, kt % 8, :],
                                start=(kt == 0),
                                stop=(kt == DKT - 1),
                            )
                        yield 8 * 216.0
                    raw = tmp.tile([128, QB], fp16, tag="tmp")
                    nc.scalar.copy(raw[:], pss[:])
                    credits["act"] += 720.0
                    sw_ps = psum.tile([128, QB], f32, tag="wo", bufs=2,
                                      name=f"dqsw{u}")
                    nc.tensor.matmul(sw_ps[:], sw_sb[:], raw[:], start=True,
                                     stop=True)
                    t1 = t12.tile([128, QB], fp16, tag="t12")
                    nc.vector.tensor_mul(t1[:], raw[:], rc_sb[:, q0:q0 + QB])
                    t2 = t12.tile([128, QB], fp16, tag="t12")
                    nc.vector.tensor_mul(t2[:], sw_ps[:], rs_sb[:, q0:q0 + QB])
                    nc.vector.tensor_add(qt_sb[:, u, q0:q0 + QB], t1[:], t2[:])
                    yield 216.0

            def gen_wo(qi):
                # wo projection for query quarter qi (at_sb rows complete)
                for sti in range(4):
                    st = qi * 4 + sti
                    o_sb = outp.tile([128, DIM], fp16, tag="outp",
                                     name=f"o_{st}")
                    for nb in range(8):
                        wo_ps = psum.tile([128, 512], f32, tag="wo", bufs=2,
                                          name=f"wo_{st}_{nb}")
                        for h in range(HPC):
                            nc.tensor.matmul(
                                wo_ps[:],
                                at_sb[:, h, st * 128:(st + 1) * 128],
                                wo_sb[:, h, nb * 512:(nb + 1) * 512],
                                start=(h == 0),
                                stop=(h == HPC - 1),
                            )
                        nc.vector.tensor_copy(o_sb[:, nb * 512:(nb + 1) * 512],
                                              wo_ps[:])
                        yield 4 * 216.0
                    nc.gpsimd.dma_start(out_r[st], o_sb[:])

            filler = deque([gen_proj()])

            def pump():
                while filler and credits["pe"] < credits["act"] + PE_MARGIN:
                    try:
                        credits["pe"] += next(filler[0])
                    except StopIteration:
                        filler.popleft()

            def drain_all():
                while filler:
                    try:
                        next(filler[0])
                    except StopIteration:
                        filler.popleft()

            def emit_pv(p0, p1, pkp, accs, h, qi, q0, closing):
                if not closing:
                    for j, pp in ((0, p0), (1, p1)):
                        kt = 2 * pkp + j
                        for qs in range(4):
                            acc = accs[qs // 2]
                            base = (qs % 2) * 129
                            nc.tensor.matmul(
                                acc[:, base:base + 129],
                                pp[:, (qs % 4) * 128:(qs % 4 + 1) * 128],
                                va_sb[:, kt, 0:129],
                                start=(kt == 0 and qs % 2 == 0),
                                stop=False,
                            )
                else:
                    # qs-major on the final pair: each accumulator pair
                    # closes as early as possible for its divide
                    for pair in range(2):
                        acc = accs[pair]
                        for sub in range(2):
                            qs = pair * 2 + sub
                            base = sub * 129
                            for j, pp in ((0, p0), (1, p1)):
                                kt = 2 * pkp + j
                                nc.tensor.matmul(
                                    acc[:, base:base + 129],
                                    pp[:, qs * 128:(qs + 1) * 128],
                                    va_sb[:, kt, 0:129],
                                    start=False,
                                    stop=(kt == KT - 1),
                                )
                        emit_divide(acc, h, q0, pair)
                    if h == HPC - 1:
                        filler.append(gen_wo(qi))

            def emit_divide(acc, h, q0, pair):
                # normalize (on DVE only; keeps ACT exp stream and PE unblocked)
                for sub in range(2):
                    qs = pair * 2 + sub
                    base = sub * 129
                    linv = small.tile([128, 1], f32, tag="small")
                    nc.vector.reciprocal(linv[:], acc[:, base + 128:base + 129])
                    a_sb = asbp.tile([128, 128], fp16, tag="asb")
                    nc.vector.tensor_scalar_mul(a_sb[:], acc[:, base:base + 128],
                                                linv[:, 0:1])
                    # sync queue only: a scalar-queue transpose would ride the
                    # ACT sequencer and stall the exp stream
                    nc.sync.dma_start_transpose(
                        at_sb[:, h, q0 + qs * 128:q0 + (qs + 1) * 128], a_sb[:]
                    )

            prev = None
            for qi in range(NQB):
                q0 = qi * QB
                for h in range(HPC):
                    accA = psum.tile([128, 258], f32, tag="accA", bufs=2,
                                     name=f"accA_{qi}_{h}")
                    accB = psum.tile([128, 258], f32, tag="accB", bufs=2,
                                     name=f"accB_{qi}_{h}")
                    for kp in range(NPAIR):
                        s_ps = psum.tile([128, 1024], f32, tag="s", name="s_ps")
                        for j in range(2):
                            kt = 2 * kp + j
                            nc.tensor.matmul(
                                s_ps[:, j * QB:(j + 1) * QB],
                                kt_sb[:, kt * 128:(kt + 1) * 128],
                                qt_sb[:, h, q0:q0 + QB],
                                start=True,
                                stop=True,
                            )
                        credits["pe"] += PE_S_NS
                        p0 = ptp.tile([128, 512], fp16, tag="pt", name="p0")
                        p1 = ptp.tile([128, 512], fp16, tag="pt", name="p1")
                        nc.scalar.activation(p0[:], s_ps[:, 0:512], AF.Exp,
                                             bias=ebias_sb[:, 0:1], scale=SCALE)
                        nc.scalar.activation(p1[:], s_ps[:, 512:1024], AF.Exp,
                                             bias=ebias_sb[:, 0:1], scale=SCALE)
                        credits["act"] += ACT_PAIR_NS
                        if prev is not None:
                            emit_pv(*prev)
                            credits["pe"] += PE_PV_NS
                        pump()
                        prev = (p0, p1, kp, (accA, accB), h, qi, q0,
                                kp == NPAIR - 1)
            # drain: last head's PV + divide, then whatever filler remains
            emit_pv(*prev)
            drain_all()

    nc.compile()
    return nc


def _get_nc():
    if "nc" not in _CACHE:
        _CACHE["nc"] = _build_nc()
    return _CACHE["nc"]


def _make_in_maps(x, freqs_cos, freqs_sin, wq, wk, wv, wo):
    x = np.asarray(x, dtype=np.float32)
    freqs_cos = np.asarray(freqs_cos, dtype=np.float32)
    freqs_sin = np.asarray(freqs_sin, dtype=np.float32)
    wq = np.asarray(wq, dtype=np.float32)
    wk = np.asarray(wk, dtype=np.float32)
    wv = np.asarray(wv, dtype=np.float32)
    wo = np.asarray(wo, dtype=np.float32)
    xt = np.ascontiguousarray(x.T).astype(np.float16)
    rc = np.repeat(freqs_cos.T, 2, axis=0).astype(np.float16)
    sgn = np.where(np.arange(HD) % 2 == 0, -1.0, 1.0)[:, None].astype(np.float32)
    rs = (np.repeat(freqs_sin.T, 2, axis=0) * sgn).astype(np.float16)
    sw = np.zeros((HD, HD), np.float16)
    idx = np.arange(HD)
    sw[idx, idx ^ 1] = 1.0
    in_maps = []
    for c in range(NCORES):
        in_maps.append({
            "xt": xt,
            "wq": np.ascontiguousarray(wq[:, c * 512:(c + 1) * 512]).astype(np.float16),
            "wkv": np.ascontiguousarray(np.concatenate(
                [wk[:, c * 128:(c + 1) * 128], wv[:, c * 128:(c + 1) * 128]],
                axis=1)).astype(np.float16),
            "wo": np.ascontiguousarray(wo[c * 512:(c + 1) * 512, :]).astype(np.float16),
            "ropec": rc,
            "ropes": rs,
            "pswap": sw,
        })
    return in_maps


def _run(inputs, trace=False):
    from concourse.bass_utils import run_bass_kernel_spmd

    nc = _get_nc()
    in_maps = _make_in_maps(**inputs)
    res = run_bass_kernel_spmd(nc, in_maps, core_ids=list(range(NCORES)), trace=trace)
    parts = [r["out"].astype(np.float32) for r in res.results]
    out = np.sum(np.stack(parts), axis=0)
    return out, res


def kernel(**inputs) -> np.ndarray:
    out, _ = _run(inputs, trace=False)
    return out


# revision 7
# speedup vs baseline: 1.0931x; 1.0931x over previous
"""Trainium2 Bass kernel: GQA attention block (nn_Attention_66142496358763).

Full module: x -> (wq,wk,wv) projections -> RoPE(q,k) -> softmax(q k^T/sqrt(d)) v
(GQA: 32 q heads, 8 kv heads) -> wo projection.

Sharding (tensor-parallel over heads, 8 cores):
  core c: q heads [4c, 4c+4), kv head c, wq/wk/wv column shards, wo row shard
  -> each core emits a partial [S, DIM] output; host sums the 8 partials.

All TensorE math in fp16 (full-rate on trn2), fp32 PSUM accumulation,
softmax exp in fp32 on ScalarE. Softmax is computed without the max
subtraction (scores are O(10) here; a -4 bias inside exp keeps the fp16
P-matrix in range) and the denominator comes for free from a ones-column
appended to V inside the PV matmul. A/V transposes ride the DMA xbar.

Schedule: the ScalarE exp stream (~1.4us per key-tile pair) outpaces the
attention-only PE work (~0.9us per pair), so the wo-projection matmuls and
the deferred quarter-3 Q projections are interleaved into the attention
inner loop as filler, paced by a credit model so the PE never waits on the
exp->PV dependency. PV accumulators are packed two-per-PSUM-bank and
double-buffered across heads so the divide of head h never blocks head h+1.
"""

import numpy as np
from collections import deque

S = 2048
DIM = 4096
HD = 128
NCORES = 8
HPC = 4          # q heads per core
QB = 512         # q block (seq block) size
NQB = S // QB    # 4
DKT = DIM // 128  # 32 contraction tiles for projections
KT = S // 128    # 16 key tiles for attention
NPAIR = KT // 2  # 8 score/exp pairs per (head, qblock)
SCALE = float(HD) ** -0.5
EXP_BIAS = -4.0
N_WARM = 120

# credit-model constants (ns of engine time per emitted op)
ACT_PAIR_NS = 1250.0   # one 1024-wide exp + semaphore handling per kp pair
PE_S_NS = 432.0        # two 512-col score matmuls
PE_PV_NS = 456.0       # eight 129-col PV matmuls
PE_MARGIN = 500.0
PRIME_NS = 2000.0      # initial act credit: builds PE queue depth up front
FRESH_KP = 2           # kp steps before a new filler generator may be pumped

_CACHE = {}


def _build_nc():
    import concourse.bass as bass
    import concourse.tile as tile
    from concourse import bacc, mybir

    fp16 = mybir.dt.float16
    f32 = mybir.dt.float32
    AF = mybir.ActivationFunctionType

    nc = bacc.Bacc("TRN2", target_bir_lowering=False, debug=False)

    xt_d = nc.dram_tensor("xt", [DIM, S], fp16, kind="ExternalInput").ap()
    wq_d = nc.dram_tensor("wq", [DIM, HPC * HD], fp16, kind="ExternalInput").ap()
    wkv_d = nc.dram_tensor("wkv", [DIM, 2 * HD], fp16, kind="ExternalInput").ap()
    wo_d = nc.dram_tensor("wo", [HPC * HD, DIM], fp16, kind="ExternalInput").ap()
    rc_d = nc.dram_tensor("ropec", [HD, S], fp16, kind="ExternalInput").ap()
    rs_d = nc.dram_tensor("ropes", [HD, S], fp16, kind="ExternalInput").ap()
    sw_d = nc.dram_tensor("pswap", [HD, HD], fp16, kind="ExternalInput").ap()
    out_d = nc.dram_tensor("out", [S, DIM], fp16, kind="ExternalOutput").ap()

    xt_r = xt_d.rearrange("(kt p) s -> p kt s", p=128)
    wq_r = wq_d.rearrange("(kt p) n -> p kt n", p=128)
    wkv_r = wkv_d.rearrange("(kt p) n -> p kt n", p=128)
    wo_r = wo_d.rearrange("(h p) n -> p h n", p=128)
    out_r = out_d.rearrange("(st p) n -> st p n", p=128)

    with tile.TileContext(nc) as tc:
        with (
            tc.tile_pool(name="const", bufs=1) as const,
            tc.tile_pool(name="xtp", bufs=5) as xtp,
            tc.tile_pool(name="persist", bufs=1) as persist,
            tc.tile_pool(name="tmp", bufs=5) as tmp,
            tc.tile_pool(name="t12", bufs=4) as t12,
            tc.tile_pool(name="pt", bufs=4) as ptp,
            tc.tile_pool(name="asb", bufs=6) as asbp,
            tc.tile_pool(name="small", bufs=8) as small,
            tc.tile_pool(name="outp", bufs=2) as outp,
            tc.tile_pool(name="psum", bufs=1, space="PSUM") as psum,
        ):
            # ---- inputs split across the two HWDGE queues:
            # sync: wk + the xt chunk stream; scalar: rope consts, wv, wq, wo
            xt_tiles = {0: []}
            for j in range(4):
                t = xtp.tile([128, 8, QB], fp16, tag="xt", name=f"xt_0_{j}")
                nc.sync.dma_start(t[:], xt_r[:, 8 * j:8 * (j + 1), 0:QB])
                xt_tiles[0].append(t)
            wkv_c = []
            wq_c = []
            for j in range(4):
                w = const.tile([128, 8, 2 * HD], fp16, tag=f"c_wkv{j}", name=f"wkv_c{j}")
                nc.scalar.dma_start(w[:], wkv_r[:, 8 * j:8 * (j + 1), :])
                wkv_c.append(w)
                q = const.tile([128, 8, 512], fp16, tag=f"c_wq{j}", name=f"wq_c{j}")
                nc.scalar.dma_start(q[:], wq_r[:, 8 * j:8 * (j + 1), :])
                wq_c.append(q)
            rc_sb = const.tile([HD, S], fp16, tag="c_rc")
            nc.scalar.dma_start(rc_sb[:], rc_d[:])
            rs_sb = const.tile([HD, S], fp16, tag="c_rs")
            nc.scalar.dma_start(rs_sb[:], rs_d[:])
            sw_sb = const.tile([HD, HD], fp16, tag="c_sw")
            nc.scalar.dma_start(sw_sb[:], sw_d[:])
            ebias_sb = const.tile([128, 1], f32, tag="c_eb")
            nc.gpsimd.memset(ebias_sb[:], EXP_BIAS)

            # PE warm-up: dummy matmuls with no input deps, so the HAM
            # clock-gate opens while the first DMAs are still streaming in
            warm_sb = const.tile([128, 128], fp16, tag="c_warm")
            nc.gpsimd.memset(warm_sb[:], 0.0)
            warm_ps = psum.tile([128, 1024], f32, tag="s", name="warm_ps")
            for _ in range(N_WARM):
                nc.tensor.matmul(
                    warm_ps[:, 0:128], warm_sb[:], warm_sb[:], start=True, stop=True
                )

            # persistent activations
            qt_sb = persist.tile([128, HPC, S], fp16, tag="p_qt")   # rope'd Q^T per head
            kt_sb = persist.tile([128, S], fp16, tag="p_kt")        # rope'd K^T
            va_sb = persist.tile([128, KT, 256], fp16, tag="p_va")  # V natural + ones col (256B-aligned rows for the xbar transpose)
            at_sb = persist.tile([128, HPC, S], fp16, tag="p_at")   # A^T per head
            nc.gpsimd.memset(va_sb[:, :, 128:130], 1.0)

            wo_sb = const.tile([128, HPC, DIM], fp16, tag="c_wo")

            # ---- phase 1: projections + rope, one seq-quarter at a time ----
            # unit order per quarter: K, V, Q0..Q3; quarter 3 only projects
            # K and V here — its four Q units are deferred into the attention
            # phase as PE filler (their xt tiles are the last ones allocated
            # from the pool, so they stay resident).
            pending = []

            def finish_unit(kind, raw, q0, u):
                if kind == "v":
                    kt0 = q0 // 128
                    for j in range(4):
                        nc.scalar.dma_start_transpose(
                            va_sb[:, kt0 + j, 0:128],
                            raw[:, j * 128:(j + 1) * 128],
                        )
                else:
                    sw_ps = psum.tile([128, QB], f32, tag="s", name="sw_ps")
                    nc.tensor.matmul(sw_ps[:], sw_sb[:], raw[:], start=True, stop=True)
                    t1 = t12.tile([128, QB], fp16, tag="t12")
                    nc.vector.tensor_mul(t1[:], raw[:], rc_sb[:, q0:q0 + QB])
                    t2 = t12.tile([128, QB], fp16, tag="t12")
                    nc.vector.tensor_mul(t2[:], sw_ps[:], rs_sb[:, q0:q0 + QB])
                    if kind == "q":
                        dest = qt_sb[:, u, q0:q0 + QB]
                    else:
                        dest = kt_sb[:, q0:q0 + QB]
                    nc.vector.tensor_add(dest, t1[:], t2[:])

            def prefetch_xt(qi, j):
                if qi >= NQB:
                    return
                lst = xt_tiles.setdefault(qi, [])
                if len(lst) > j:
                    return
                nq0 = qi * QB
                t = xtp.tile([128, 8, QB], fp16, tag="xt", name=f"xt_{qi}_{j}")
                nc.sync.dma_start(t[:], xt_r[:, 8 * j:8 * (j + 1), nq0:nq0 + QB])
                lst.append(t)

            def w_slice(kind, h, kt):
                if kind == "q":
                    return wq_c[kt // 8][:, kt % 8, h * HD:(h + 1) * HD]
                if kind == "k":
                    return wkv_c[kt // 8][:, kt % 8, 0:HD]
                return wkv_c[kt // 8][:, kt % 8, HD:2 * HD]

            PS_TAGS = ["accA", "accB", "wo"]

            for qi in range(NQB):
                q0 = qi * QB
                xt_c = xt_tiles[qi]
                if qi < NQB - 1:
                    groups = [[("k", -1), ("v", -1), ("q", 0)],
                              [("q", 1), ("q", 2), ("q", 3)]]
                else:
                    groups = [[("k", -1), ("v", -1)]]
                for gi, grp in enumerate(groups):
                    if gi == 0:
                        prefetch_xt(qi + 1, 0)
                        prefetch_xt(qi + 1, 1)
                    else:
                        prefetch_xt(qi + 1, 2)
                        prefetch_xt(qi + 1, 3)
                    pss = [
                        psum.tile([128, QB], f32, tag=PS_TAGS[gu], bufs=2,
                                  name=f"pj{gi}{gu}")
                        for gu in range(len(grp))
                    ]
                    for j in range(4):
                        for gu, (kind, h) in enumerate(grp):
                            for kt in range(8 * j, 8 * j + 8):
                                nc.tensor.matmul(
                                    pss[gu][:],
                                    w_slice(kind, h, kt),
                                    xt_c[kt // 8][:, kt % 8, :],
                                    start=(kt == 0),
                                    stop=(kt == DKT - 1),
                                )
                        if pending:
                            finish_unit(*pending.pop(0))
                    for gu, (kind, h) in enumerate(grp):
                        raw = tmp.tile([128, QB], fp16, tag="tmp")
                        nc.scalar.copy(raw[:], pss[gu][:])
                        pending.append((kind, raw, q0, h))
            while pending:
                finish_unit(*pending.pop(0))

            nc.scalar.dma_start(wo_sb[:], wo_r[:])

            # ---- phase 2+3: attention with interleaved filler ----
            credits = {"pe": 0.0, "act": 0.0}

            def gen_proj():
                # deferred quarter-3 Q projections (rope'd into qt_sb)
                xt_c = xt_tiles[NQB - 1]
                q0 = (NQB - 1) * QB
                for u in range(HPC):
                    pss = psum.tile([128, QB], f32, tag="wo", bufs=2,
                                    name=f"dq{u}")
                    for j in range(4):
                        for kt in range(8 * j, 8 * j + 8):
                            nc.tensor.matmul(
                                pss[:],
                                wq_c[kt // 8][:, kt % 8, u * HD:(u + 1) * HD],
                                xt_c[kt // 8][:, kt % 8, :],
                                start=(kt == 0),
                                stop=(kt == DKT - 1),
                            )
                        yield 8 * 216.0
                    raw = tmp.tile([128, QB], fp16, tag="tmp")
                    nc.scalar.copy(raw[:], pss[:])
                    credits["act"] += 720.0
                    sw_ps = psum.tile([128, QB], f32, tag="wo", bufs=2,
                                      name=f"dqsw{u}")
                    nc.tensor.matmul(sw_ps[:], sw_sb[:], raw[:], start=True,
                                     stop=True)
                    t1 = t12.tile([128, QB], fp16, tag="t12")
                    nc.vector.tensor_mul(t1[:], raw[:], rc_sb[:, q0:q0 + QB])
                    t2 = t12.tile([128, QB], fp16, tag="t12")
                    nc.vector.tensor_mul(t2[:], sw_ps[:], rs_sb[:, q0:q0 + QB])
                    nc.vector.tensor_add(qt_sb[:, u, q0:q0 + QB], t1[:], t2[:])
                    yield 216.0

            def gen_wo(qi):
                # wo projection for query quarter qi (at_sb rows complete)
                for sti in range(4):
                    st = qi * 4 + sti
                    o_sb = outp.tile([128, DIM], fp16, tag="outp",
                                     name=f"o_{st}")
                    for nb in range(8):
                        wo_ps = psum.tile([128, 512], f32, tag="wo", bufs=2,
                                          name=f"wo_{st}_{nb}")
                        for h in range(HPC):
                            nc.tensor.matmul(
                                wo_ps[:],
                                at_sb[:, h, st * 128:(st + 1) * 128],
                                wo_sb[:, h, nb * 512:(nb + 1) * 512],
                                start=(h == 0),
                                stop=(h == HPC - 1),
                            )
                        nc.vector.tensor_copy(o_sb[:, nb * 512:(nb + 1) * 512],
                                              wo_ps[:])
                        yield 4 * 216.0
                    nc.gpsimd.dma_start(out_r[st], o_sb[:])

            filler = deque([[gen_proj(), -100]])
            step = [0]

            def pump():
                while filler and credits["pe"] < credits["act"] + PE_MARGIN:
                    g, born = filler[0]
                    if step[0] - born < FRESH_KP:
                        break  # deps of a fresh generator are still in flight
                    try:
                        credits["pe"] += next(g)
                    except StopIteration:
                        filler.popleft()

            def drain_all():
                while filler:
                    try:
                        next(filler[0][0])
                    except StopIteration:
                        filler.popleft()

            def emit_pv(pp, pkp, accs, h, qi, q0, closing):
                if not closing:
                    for j in range(2):
                        kt = 2 * pkp + j
                        for qs in range(4):
                            acc = accs[qs // 2]
                            base = (qs % 2) * 129
                            nc.tensor.matmul(
                                acc[:, base:base + 129],
                                pp[:, j * QB + qs * 128:j * QB + (qs + 1) * 128],
                                va_sb[:, kt, 0:129],
                                start=(kt == 0 and qs % 2 == 0),
                                stop=False,
                            )
                else:
                    # qs-major on the final pair: each accumulator pair
                    # closes as early as possible for its divide
                    for pair in range(2):
                        acc = accs[pair]
                        for sub in range(2):
                            qs = pair * 2 + sub
                            base = sub * 129
                            for j in range(2):
                                kt = 2 * pkp + j
                                nc.tensor.matmul(
                                    acc[:, base:base + 129],
                                    pp[:, j * QB + qs * 128:j * QB + (qs + 1) * 128],
                                    va_sb[:, kt, 0:129],
                                    start=False,
                                    stop=(kt == KT - 1),
                                )
                        emit_divide(acc, h, q0, pair)
                    if h == HPC - 1:
                        filler.append([gen_wo(qi), step[0]])

            def emit_divide(acc, h, q0, pair):
                # normalize (on DVE only; keeps ACT exp stream and PE unblocked)
                for sub in range(2):
                    qs = pair * 2 + sub
                    base = sub * 129
                    linv = small.tile([128, 1], f32, tag="small")
                    nc.vector.reciprocal(linv[:], acc[:, base + 128:base + 129])
                    a_sb = asbp.tile([128, 128], fp16, tag="asb")
                    nc.vector.tensor_scalar_mul(a_sb[:], acc[:, base:base + 128],
                                                linv[:, 0:1])
                    # sync queue only: a scalar-queue transpose would ride the
                    # ACT sequencer and stall the exp stream
                    nc.sync.dma_start_transpose(
                        at_sb[:, h, q0 + qs * 128:q0 + (qs + 1) * 128], a_sb[:]
                    )

            credits["act"] += PRIME_NS
            prev = None
            for qi in range(NQB):
                q0 = qi * QB
                for h in range(HPC):
                    accA = psum.tile([128, 258], f32, tag="accA", bufs=2,
                                     name=f"accA_{qi}_{h}")
                    accB = psum.tile([128, 258], f32, tag="accB", bufs=2,
                                     name=f"accB_{qi}_{h}")
                    for kp in range(NPAIR):
                        step[0] += 1
                        s_ps = psum.tile([128, 1024], f32, tag="s", name="s_ps")
                        for j in range(2):
                            kt = 2 * kp + j
                            nc.tensor.matmul(
                                s_ps[:, j * QB:(j + 1) * QB],
                                kt_sb[:, kt * 128:(kt + 1) * 128],
                                qt_sb[:, h, q0:q0 + QB],
                                start=True,
                                stop=True,
                            )
                        credits["pe"] += PE_S_NS
                        p_t = ptp.tile([128, 1024], fp16, tag="pt", name="p_t")
                        nc.scalar.activation(p_t[:], s_ps[:], AF.Exp,
                                             bias=ebias_sb[:, 0:1], scale=SCALE)
                        credits["act"] += ACT_PAIR_NS
                        if prev is not None:
                            emit_pv(*prev)
                            credits["pe"] += PE_PV_NS
                        pump()
                        prev = (p_t, kp, (accA, accB), h, qi, q0,
                                kp == NPAIR - 1)
            # drain: last head's PV + divide, then whatever filler remains
            emit_pv(*prev)
            drain_all()

    nc.compile()
    return nc


def _get_nc():
    if "nc" not in _CACHE:
        _CACHE["nc"] = _build_nc()
    return _CACHE["nc"]


def _make_in_maps(x, freqs_cos, freqs_sin, wq, wk, wv, wo):
    x = np.asarray(x, dtype=np.float32)
    freqs_cos = np.asarray(freqs_cos, dtype=np.float32)
    freqs_sin = np.asarray(freqs_sin, dtype=np.float32)
    wq = np.asarray(wq, dtype=np.float32)
    wk = np.asarray(wk, dtype=np.float32)
    wv = np.asarray(wv, dtype=np.float32)
    wo = np.asarray(wo, dtype=np.float32)
    xt = np.ascontiguousarray(x.T).astype(np.float16)
    rc = np.repeat(freqs_cos.T, 2, axis=0).astype(np.float16)
    sgn = np.where(np.arange(HD) % 2 == 0, -1.0, 1.0)[:, None].astype(np.float32)
    rs = (np.repeat(freqs_sin.T, 2, axis=0) * sgn).astype(np.float16)
    sw = np.zeros((HD, HD), np.float16)
    idx = np.arange(HD)
    sw[idx, idx ^ 1] = 1.0
    in_maps = []
    for c in range(NCORES):
        in_maps.append({
            "xt": xt,
            "wq": np.ascontiguousarray(wq[:, c * 512:(c + 1) * 512]).astype(np.float16),
            "wkv": np.ascontiguousarray(np.concatenate(
                [wk[:, c * 128:(c + 1) * 128], wv[:, c * 128:(c + 1) * 128]],
                axis=1)).astype(np.float16),
            "wo": np.ascontiguousarray(wo[c * 512:(c + 1) * 512, :]).astype(np.float16),
            "ropec": rc,
            "ropes": rs,
            "pswap": sw,
        })
    return in_maps


def _run(inputs, trace=False):
    from concourse.bass_utils import run_bass_kernel_spmd

    nc = _get_nc()
    in_maps = _make_in_maps(**inputs)
    res = run_bass_kernel_spmd(nc, in_maps, core_ids=list(range(NCORES)), trace=trace)
    parts = [r["out"].astype(np.float32) for r in res.results]
    out = np.sum(np.stack(parts), axis=0)
    return out, res


def kernel(**inputs) -> np.ndarray:
    out, _ = _run(inputs, trace=False)
    return out


# revision 9
# speedup vs baseline: 1.0999x; 1.0062x over previous
"""Trainium2 Bass kernel: GQA attention block (nn_Attention_66142496358763).

Full module: x -> (wq,wk,wv) projections -> RoPE(q,k) -> softmax(q k^T/sqrt(d)) v
(GQA: 32 q heads, 8 kv heads) -> wo projection.

Sharding (tensor-parallel over heads, 8 cores):
  core c: q heads [4c, 4c+4), kv head c, wq/wk/wv column shards, wo row shard
  -> each core emits a partial [S, DIM] output; host sums the 8 partials.

All TensorE math in fp16 (full-rate on trn2), fp32 PSUM accumulation,
softmax exp in fp32 on ScalarE. Softmax is computed without the max
subtraction (scores are O(10) here; a -4 bias inside exp keeps the fp16
P-matrix in range) and the denominator comes for free from a ones-column
appended to V inside the PV matmul. A/V transposes ride the DMA xbar.

Schedule: the ScalarE exp stream (~1.4us per key-tile pair) outpaces the
attention-only PE work (~0.9us per pair), so the wo-projection matmuls and
the deferred quarter-3 Q projections are interleaved into the attention
inner loop as filler, paced by a credit model so the PE never waits on the
exp->PV dependency. PV accumulators are packed two-per-PSUM-bank and
double-buffered across heads so the divide of head h never blocks head h+1.
"""

import numpy as np
from collections import deque

S = 2048
DIM = 4096
HD = 128
NCORES = 8
HPC = 4          # q heads per core
QB = 512         # q block (seq block) size
NQB = S // QB    # 4
DKT = DIM // 128  # 32 contraction tiles for projections
KT = S // 128    # 16 key tiles for attention
NPAIR = KT // 2  # 8 score/exp pairs per (head, qblock)
SCALE = float(HD) ** -0.5
EXP_BIAS = -4.0
N_WARM = 120

# credit-model constants (ns of engine time per emitted op)
ACT_PAIR_NS = 1250.0   # one 1024-wide exp + semaphore handling per kp pair
PE_S_NS = 432.0        # two 512-col score matmuls
PE_PV_NS = 456.0       # eight 129-col PV matmuls
PE_MARGIN = 500.0
PRIME_NS = 2000.0      # initial act credit: builds PE queue depth up front
FRESH_KP = 2           # kp steps before a new filler generator may be pumped

_CACHE = {}


def _build_nc():
    import concourse.bass as bass
    import concourse.tile as tile
    from concourse import bacc, mybir

    fp16 = mybir.dt.float16
    f32 = mybir.dt.float32
    AF = mybir.ActivationFunctionType

    nc = bacc.Bacc("TRN2", target_bir_lowering=False, debug=False)

    xt_d = nc.dram_tensor("xt", [DIM, S], fp16, kind="ExternalInput").ap()
    wq_d = nc.dram_tensor("wq", [DIM, HPC * HD], fp16, kind="ExternalInput").ap()
    wkv_d = nc.dram_tensor("wkv", [DIM, 2 * HD], fp16, kind="ExternalInput").ap()
    wo_d = nc.dram_tensor("wo", [HPC * HD, DIM], fp16, kind="ExternalInput").ap()
    rc_d = nc.dram_tensor("ropec", [HD, S], fp16, kind="ExternalInput").ap()
    rs_d = nc.dram_tensor("ropes", [HD, S], fp16, kind="ExternalInput").ap()
    sw_d = nc.dram_tensor("pswap", [HD, HD], fp16, kind="ExternalInput").ap()
    out_d = nc.dram_tensor("out", [S, DIM], fp16, kind="ExternalOutput").ap()

    xt_r = xt_d.rearrange("(kt p) s -> p kt s", p=128)
    wq_r = wq_d.rearrange("(kt p) n -> p kt n", p=128)
    wkv_r = wkv_d.rearrange("(kt p) n -> p kt n", p=128)
    wo_r = wo_d.rearrange("(h p) n -> p h n", p=128)
    out_r = out_d.rearrange("(st p) n -> st p n", p=128)

    with tile.TileContext(nc) as tc:
        with (
            tc.tile_pool(name="const", bufs=1) as const,
            tc.tile_pool(name="xtp", bufs=5) as xtp,
            tc.tile_pool(name="persist", bufs=1) as persist,
            tc.tile_pool(name="tmp", bufs=5) as tmp,
            tc.tile_pool(name="t12", bufs=4) as t12,
            tc.tile_pool(name="pt", bufs=4) as ptp,
            tc.tile_pool(name="asb", bufs=6) as asbp,
            tc.tile_pool(name="small", bufs=8) as small,
            tc.tile_pool(name="outp", bufs=2) as outp,
            tc.tile_pool(name="psum", bufs=1, space="PSUM") as psum,
        ):
            # ---- inputs split across the two HWDGE queues:
            # sync: wk + the xt chunk stream; scalar: rope consts, wv, wq, wo
            xt_tiles = {0: []}
            for j in range(4):
                t = xtp.tile([128, 8, QB], fp16, tag="xt", name=f"xt_0_{j}")
                nc.sync.dma_start(t[:], xt_r[:, 8 * j:8 * (j + 1), 0:QB])
                xt_tiles[0].append(t)
            wkv_c = []
            wq_c = []
            for j in range(4):
                w = const.tile([128, 8, 2 * HD], fp16, tag=f"c_wkv{j}", name=f"wkv_c{j}")
                nc.scalar.dma_start(w[:], wkv_r[:, 8 * j:8 * (j + 1), :])
                wkv_c.append(w)
                q = const.tile([128, 8, 512], fp16, tag=f"c_wq{j}", name=f"wq_c{j}")
                nc.scalar.dma_start(q[:], wq_r[:, 8 * j:8 * (j + 1), :])
                wq_c.append(q)
            rc_sb = const.tile([HD, S], fp16, tag="c_rc")
            nc.scalar.dma_start(rc_sb[:], rc_d[:])
            rs_sb = const.tile([HD, S], fp16, tag="c_rs")
            nc.scalar.dma_start(rs_sb[:], rs_d[:])
            sw_sb = const.tile([HD, HD], fp16, tag="c_sw")
            nc.scalar.dma_start(sw_sb[:], sw_d[:])
            ebias_sb = const.tile([128, 1], f32, tag="c_eb")
            nc.gpsimd.memset(ebias_sb[:], EXP_BIAS)

            # PE warm-up: dummy matmuls with no input deps, so the HAM
            # clock-gate opens while the first DMAs are still streaming in
            warm_sb = const.tile([128, 128], fp16, tag="c_warm")
            nc.gpsimd.memset(warm_sb[:], 0.0)
            warm_ps = psum.tile([128, 1024], f32, tag="s", name="warm_ps")
            for _ in range(N_WARM):
                nc.tensor.matmul(
                    warm_ps[:, 0:128], warm_sb[:], warm_sb[:], start=True, stop=True
                )

            # persistent activations
            qt_sb = persist.tile([128, HPC, S], fp16, tag="p_qt")   # rope'd Q^T per head
            kt_sb = persist.tile([128, S], fp16, tag="p_kt")        # rope'd K^T
            va_sb = persist.tile([128, KT, 256], fp16, tag="p_va")  # V natural + ones col (256B-aligned rows for the xbar transpose)
            at_sb = persist.tile([128, HPC, S], fp16, tag="p_at")   # A^T per head
            nc.gpsimd.memset(va_sb[:, :, 128:130], 1.0)

            wo_sb = const.tile([128, HPC, DIM], fp16, tag="c_wo")

            # ---- phase 1: projections + rope, one seq-quarter at a time ----
            # unit order per quarter: K, V, Q0..Q3; quarter 3 only projects
            # K and V here — its four Q units are deferred into the attention
            # phase as PE filler (their xt tiles are the last ones allocated
            # from the pool, so they stay resident).
            pending = []

            def finish_unit(kind, raw, q0, u):
                if kind == "v":
                    kt0 = q0 // 128
                    for j in range(4):
                        nc.scalar.dma_start_transpose(
                            va_sb[:, kt0 + j, 0:128],
                            raw[:, j * 128:(j + 1) * 128],
                        )
                else:
                    sw_ps = psum.tile([128, QB], f32, tag="s", name="sw_ps")
                    nc.tensor.matmul(sw_ps[:], sw_sb[:], raw[:], start=True, stop=True)
                    t1 = t12.tile([128, QB], fp16, tag="t12")
                    nc.vector.tensor_mul(t1[:], raw[:], rc_sb[:, q0:q0 + QB])
                    t2 = t12.tile([128, QB], fp16, tag="t12")
                    nc.vector.tensor_mul(t2[:], sw_ps[:], rs_sb[:, q0:q0 + QB])
                    if kind == "q":
                        dest = qt_sb[:, u, q0:q0 + QB]
                    else:
                        dest = kt_sb[:, q0:q0 + QB]
                    nc.vector.tensor_add(dest, t1[:], t2[:])

            def prefetch_xt(qi, j):
                if qi >= NQB:
                    return
                lst = xt_tiles.setdefault(qi, [])
                if len(lst) > j:
                    return
                nq0 = qi * QB
                t = xtp.tile([128, 8, QB], fp16, tag="xt", name=f"xt_{qi}_{j}")
                nc.sync.dma_start(t[:], xt_r[:, 8 * j:8 * (j + 1), nq0:nq0 + QB])
                lst.append(t)

            def w_slice(kind, h, kt):
                if kind == "q":
                    return wq_c[kt // 8][:, kt % 8, h * HD:(h + 1) * HD]
                if kind == "k":
                    return wkv_c[kt // 8][:, kt % 8, 0:HD]
                return wkv_c[kt // 8][:, kt % 8, HD:2 * HD]

            PS_TAGS = ["accA", "accB", "wo"]

            for qi in range(NQB):
                q0 = qi * QB
                xt_c = xt_tiles[qi]
                if qi < NQB - 1:
                    groups = [[("k", -1), ("v", -1), ("q", 0)],
                              [("q", 1), ("q", 2), ("q", 3)]]
                else:
                    groups = [[("k", -1), ("v", -1)]]
                for gi, grp in enumerate(groups):
                    if gi == 0:
                        prefetch_xt(qi + 1, 0)
                        prefetch_xt(qi + 1, 1)
                    else:
                        prefetch_xt(qi + 1, 2)
                        prefetch_xt(qi + 1, 3)
                    pss = [
                        psum.tile([128, QB], f32, tag=PS_TAGS[gu], bufs=2,
                                  name=f"pj{gi}{gu}")
                        for gu in range(len(grp))
                    ]
                    for j in range(4):
                        for gu, (kind, h) in enumerate(grp):
                            for kt in range(8 * j, 8 * j + 8):
                                nc.tensor.matmul(
                                    pss[gu][:],
                                    w_slice(kind, h, kt),
                                    xt_c[kt // 8][:, kt % 8, :],
                                    start=(kt == 0),
                                    stop=(kt == DKT - 1),
                                )
                        if pending:
                            finish_unit(*pending.pop(0))
                    for gu, (kind, h) in enumerate(grp):
                        raw = tmp.tile([128, QB], fp16, tag="tmp")
                        nc.scalar.copy(raw[:], pss[gu][:])
                        pending.append((kind, raw, q0, h))
            # drain K/V first: the last K rope chain runs on the shared PSUM
            # tag and would otherwise gate the first attention score matmuls
            pending.sort(key=lambda u: u[0] == "q")
            while pending:
                finish_unit(*pending.pop(0))

            nc.scalar.dma_start(wo_sb[:], wo_r[:])

            # ---- phase 2+3: attention with interleaved filler ----
            credits = {"pe": 0.0, "act": 0.0}

            def gen_proj():
                # deferred quarter-3 Q projections (rope'd into qt_sb)
                xt_c = xt_tiles[NQB - 1]
                q0 = (NQB - 1) * QB
                for u in range(HPC):
                    pss = psum.tile([128, QB], f32, tag="wo", bufs=2,
                                    name=f"dq{u}")
                    for j in range(4):
                        for kt in range(8 * j, 8 * j + 8):
                            nc.tensor.matmul(
                                pss[:],
                                wq_c[kt // 8][:, kt % 8, u * HD:(u + 1) * HD],
                                xt_c[kt // 8][:, kt % 8, :],
                                start=(kt == 0),
                                stop=(kt == DKT - 1),
                            )
                        yield 8 * 216.0
                    raw = tmp.tile([128, QB], fp16, tag="tmp")
                    nc.scalar.copy(raw[:], pss[:])
                    credits["act"] += 720.0
                    sw_ps = psum.tile([128, QB], f32, tag="wo", bufs=2,
                                      name=f"dqsw{u}")
                    nc.tensor.matmul(sw_ps[:], sw_sb[:], raw[:], start=True,
                                     stop=True)
                    t1 = t12.tile([128, QB], fp16, tag="t12")
                    nc.vector.tensor_mul(t1[:], raw[:], rc_sb[:, q0:q0 + QB])
                    t2 = t12.tile([128, QB], fp16, tag="t12")
                    nc.vector.tensor_mul(t2[:], sw_ps[:], rs_sb[:, q0:q0 + QB])
                    nc.vector.tensor_add(qt_sb[:, u, q0:q0 + QB], t1[:], t2[:])
                    yield 216.0

            def gen_wo(qi):
                # wo projection for query quarter qi (at_sb rows complete);
                # half-row staging tiles so the WAR chain through the output
                # DMA stays several units deep
                for sti in range(4):
                    st = qi * 4 + sti
                    for half in range(2):
                        o_sb = outp.tile([128, DIM // 2], fp16, tag="outp",
                                         bufs=4, name=f"o_{st}_{half}")
                        for hb in range(4):
                            nb = half * 4 + hb
                            wo_ps = psum.tile([128, 512], f32, tag="wo", bufs=2,
                                              name=f"wo_{st}_{nb}")
                            for h in range(HPC):
                                nc.tensor.matmul(
                                    wo_ps[:],
                                    at_sb[:, h, st * 128:(st + 1) * 128],
                                    wo_sb[:, h, nb * 512:(nb + 1) * 512],
                                    start=(h == 0),
                                    stop=(h == HPC - 1),
                                )
                            nc.vector.tensor_copy(
                                o_sb[:, hb * 512:(hb + 1) * 512], wo_ps[:])
                            yield 4 * 216.0
                        nc.gpsimd.dma_start(
                            out_r[st][:, half * 2048:(half + 1) * 2048], o_sb[:])

            filler = deque([[gen_proj(), -100]])
            step = [0]

            def pump():
                while filler and credits["pe"] < credits["act"] + PE_MARGIN:
                    g, born = filler[0]
                    if step[0] - born < FRESH_KP:
                        break  # deps of a fresh generator are still in flight
                    try:
                        credits["pe"] += next(g)
                    except StopIteration:
                        filler.popleft()

            def drain_all():
                while filler:
                    try:
                        next(filler[0][0])
                    except StopIteration:
                        filler.popleft()

            def emit_pv(pp, pkp, accs, h, qi, q0, closing):
                if not closing:
                    for j in range(2):
                        kt = 2 * pkp + j
                        for qs in range(4):
                            acc = accs[qs // 2]
                            base = (qs % 2) * 129
                            nc.tensor.matmul(
                                acc[:, base:base + 129],
                                pp[:, j * QB + qs * 128:j * QB + (qs + 1) * 128],
                                va_sb[:, kt, 0:129],
                                start=(kt == 0 and qs % 2 == 0),
                                stop=False,
                            )
                else:
                    # qs-major on the final pair: each accumulator pair
                    # closes as early as possible for its divide
                    for pair in range(2):
                        acc = accs[pair]
                        for sub in range(2):
                            qs = pair * 2 + sub
                            base = sub * 129
                            for j in range(2):
                                kt = 2 * pkp + j
                                nc.tensor.matmul(
                                    acc[:, base:base + 129],
                                    pp[:, j * QB + qs * 128:j * QB + (qs + 1) * 128],
                                    va_sb[:, kt, 0:129],
                                    start=False,
                                    stop=(kt == KT - 1),
                                )
                        emit_divide(acc, h, q0, pair)
                    if h == HPC - 1:
                        filler.append([gen_wo(qi), step[0]])

            def emit_divide(acc, h, q0, pair):
                # normalize (on DVE only; keeps ACT exp stream and PE unblocked)
                for sub in range(2):
                    qs = pair * 2 + sub
                    base = sub * 129
                    linv = small.tile([128, 1], f32, tag="small")
                    nc.vector.reciprocal(linv[:], acc[:, base + 128:base + 129])
                    a_sb = asbp.tile([128, 128], fp16, tag="asb")
                    nc.vector.tensor_scalar_mul(a_sb[:], acc[:, base:base + 128],
                                                linv[:, 0:1])
                    # sync queue only: a scalar-queue transpose would ride the
                    # ACT sequencer and stall the exp stream
                    nc.sync.dma_start_transpose(
                        at_sb[:, h, q0 + qs * 128:q0 + (qs + 1) * 128], a_sb[:]
                    )

            credits["act"] += PRIME_NS
            prev = None
            for qi in range(NQB):
                q0 = qi * QB
                for h in range(HPC):
                    accA = psum.tile([128, 258], f32, tag="accA", bufs=2,
                                     name=f"accA_{qi}_{h}")
                    accB = psum.tile([128, 258], f32, tag="accB", bufs=2,
                                     name=f"accB_{qi}_{h}")
                    for kp in range(NPAIR):
                        step[0] += 1
                        s_ps = psum.tile([128, 1024], f32, tag="s", name="s_ps")
                        for j in range(2):
                            kt = 2 * kp + j
                            nc.tensor.matmul(
                                s_ps[:, j * QB:(j + 1) * QB],
                                kt_sb[:, kt * 128:(kt + 1) * 128],
                                qt_sb[:, h, q0:q0 + QB],
                                start=True,
                                stop=True,
                            )
                        credits["pe"] += PE_S_NS
                        p_t = ptp.tile([128, 1024], fp16, tag="pt", name="p_t")
                        nc.scalar.activation(p_t[:], s_ps[:], AF.Exp,
                                             bias=ebias_sb[:, 0:1], scale=SCALE)
                        credits["act"] += ACT_PAIR_NS
                        if prev is not None:
                            emit_pv(*prev)
                            credits["pe"] += PE_PV_NS
                        pump()
                        prev = (p_t, kp, (accA, accB), h, qi, q0,
                                kp == NPAIR - 1)
            # drain: last head's PV + divide, then whatever filler remains
            emit_pv(*prev)
            drain_all()

    nc.compile()
    return nc


def _get_nc():
    if "nc" not in _CACHE:
        _CACHE["nc"] = _build_nc()
    return _CACHE["nc"]


def _make_in_maps(x, freqs_cos, freqs_sin, wq, wk, wv, wo):
    x = np.asarray(x, dtype=np.float32)
    freqs_cos = np.asarray(freqs_cos, dtype=np.float32)
    freqs_sin = np.asarray(freqs_sin, dtype=np.float32)
    wq = np.asarray(wq, dtype=np.float32)
    wk = np.asarray(wk, dtype=np.float32)
    wv = np.asarray(wv, dtype=np.float32)
    wo = np.asarray(wo, dtype=np.float32)
    xt = np.ascontiguousarray(x.T).astype(np.float16)
    rc = np.repeat(freqs_cos.T, 2, axis=0).astype(np.float16)
    sgn = np.where(np.arange(HD) % 2 == 0, -1.0, 1.0)[:, None].astype(np.float32)
    rs = (np.repeat(freqs_sin.T, 2, axis=0) * sgn).astype(np.float16)
    sw = np.zeros((HD, HD), np.float16)
    idx = np.arange(HD)
    sw[idx, idx ^ 1] = 1.0
    in_maps = []
    for c in range(NCORES):
        in_maps.append({
            "xt": xt,
            "wq": np.ascontiguousarray(wq[:, c * 512:(c + 1) * 512]).astype(np.float16),
            "wkv": np.ascontiguousarray(np.concatenate(
                [wk[:, c * 128:(c + 1) * 128], wv[:, c * 128:(c + 1) * 128]],
                axis=1)).astype(np.float16),
            "wo": np.ascontiguousarray(wo[c * 512:(c + 1) * 512, :]).astype(np.float16),
            "ropec": rc,
            "ropes": rs,
            "pswap": sw,
        })
    return in_maps


def _run(inputs, trace=False):
    from concourse.bass_utils import run_bass_kernel_spmd

    nc = _get_nc()
    in_maps = _make_in_maps(**inputs)
    res = run_bass_kernel_spmd(nc, in_maps, core_ids=list(range(NCORES)), trace=trace)
    parts = [r["out"].astype(np.float32) for r in res.results]
    out = np.sum(np.stack(parts), axis=0)
    return out, res


def kernel(**inputs) -> np.ndarray:
    out, _ = _run(inputs, trace=False)
    return out
